# revision 12
# baseline (speedup 1.0000x reference)
"""Trainium2 Bass kernel for nn_Loss_56410100465732 (retrieval_knn).

reference semantics:
  x = phi_p [4,512,64,64] -> queries [16384, 512]
  d2[q,m] = clamp(||x_q||^2 + ||m_m||^2 - 2 x_q.m_m, 0)   (m over 16384 bank rows)
  dist = 6 smallest d2 per query, ascending
  loss = mean(relu(dist[:, :3] - r^2))/NU + mean(relu(r^2 - dist[:, 3:6] - ALPHA))/NU

Strategy (data-parallel over queries, 2048 queries/core on 8 cores):
  - Device computes, per query q, the top-8 LARGEST values of
      c[q,m] = dot(x_q, m_m) - 0.5*||m_m||^2
    which are exactly the 8 smallest d2 (d2 = ||x_q||^2 - 2c; the per-query
    ||x||^2 shift does not change per-query ranking).
  - PE does the dot products in bf16 (fp32 PSUM accumulate). The -0.5*||m||^2
    term is preloaded into PSUM in exact fp32 by the Scalar engine, and the
    matmuls accumulate on top (start=False).
  - The hardware top-8 instruction (nc.vector.max) runs per PSUM strip of
    2048 bank entries; per-strip top-8s are merged with a final max.
  - Host recovers d2 = ||x||^2 - 2c (fp64), applies the clamp + relus + means.
"""

import sys

if "/opt/trn_rl_repo" not in sys.path:
    sys.path.insert(0, "/opt/trn_rl_repo")

import numpy as np
import ml_dtypes

K = 3
J = 3
ALPHA = 0.1
NU = 1e-3

B, C, H, W = 4, 512, 64, 64
N_BANK = 16384
N_CORES = 8
Q_TOTAL = B * H * W            # 16384 queries
Q_PER_CORE = Q_TOTAL // N_CORES  # 2048
P = 128                        # SBUF partitions per query tile
STRIP = 2048                   # bank entries per strip (one PSUM mega-tile)
MM_N = 512                     # matmul free-dim (one PSUM bank)
KC = C // P                    # 4 contraction chunks


def build_program(qt=Q_PER_CORE // P, ns=N_BANK // STRIP, reps=1, skip_max=False, skip_mm=False):
    """SPMD program for one core: qt query-tiles of 128, ns bank strips of 2048.

    reps/skip_* are benchmarking knobs: reps repeats the compute body (marginal
    time per rep = true kernel time, cancels dispatch overhead); skip_max/skip_mm
    drop the top-8 / matmul work to isolate engine costs.
    """
    import concourse.bacc as bacc
    import concourse.mybir as mybir
    from concourse.tile import TileContext

    bf16 = mybir.dt.bfloat16
    f32 = mybir.dt.float32

    q = qt * P
    nb = ns * STRIP
    cc_per_strip = STRIP // MM_N

    nc = bacc.Bacc("TRN2", target_bir_lowering=False, debug=False, num_devices=N_CORES)
    xT = nc.declare_dram_parameter("xT", [C, q], bf16, isOutput=False)
    mT = nc.declare_dram_parameter("mT", [C, nb], bf16, isOutput=False)
    # two-row bf16 hi/lo split of -0.5*||m||^2, folded in via a contraction-2 matmul
    m2duo = nc.declare_dram_parameter("m2duo", [2, nb], bf16, isOutput=False)
    c8 = nc.declare_dram_parameter("c8", [qt, P, 8], f32, isOutput=True)

    with TileContext(nc) as tc:
        with (
            tc.tile_pool(name="xpool", bufs=1) as xpool,
            tc.tile_pool(name="mpool", bufs=2) as mpool,
            tc.tile_pool(name="spool", bufs=1) as spool,
            tc.tile_pool(name="opool", bufs=2) as opool,
            tc.tile_pool(name="ppool", bufs=2, space="PSUM") as ppool,
        ):
            # resident query chunks [128 contraction, q]
            xts = []
            for kc in range(KC):
                t = xpool.tile([P, q], bf16, tag=f"x{kc}")
                nc.sync.dma_start(out=t, in_=xT[kc * P : (kc + 1) * P, :])
                xts.append(t)

            # -0.5*||m||^2 rows + ones weights for the fold matmul
            m2sb = xpool.tile([2, nb], bf16, tag="m2sb")
            nc.sync.dma_start(out=m2sb, in_=m2duo[:, :])
            ones2 = xpool.tile([2, P], bf16, tag="ones2")
            nc.vector.memset(ones2, 1.0)

            # per-(qtile, strip) top-8 stash
            stash = None if skip_max else spool.tile([P, qt * ns * 8], f32)

            for rep in range(reps):
                for s in range(ns):
                    mts = []
                    for kc in range(KC):
                        mt_t = mpool.tile([P, STRIP], bf16, tag=f"m{kc}")
                        nc.sync.dma_start(
                            out=mt_t,
                            in_=mT[kc * P : (kc + 1) * P, s * STRIP : (s + 1) * STRIP],
                        )
                        mts.append(mt_t)
                    for t in range(qt):
                        ps = ppool.tile([P, STRIP], f32, tag="ps")
                        if skip_mm:
                            nc.vector.memset(ps[:, 0:8], 0.0)
                        if not skip_mm:
                            # kc-outer so 4 consecutive matmuls share one
                            # stationary-weight load; folds last (shared ones2
                            # weights). Groups interleave across the 4 psum
                            # bank regions, hence skip_group_check.
                            for kc in range(KC):
                                for cc in range(cc_per_strip):
                                    nc.tensor.matmul(
                                        ps[:, cc * MM_N : (cc + 1) * MM_N],
                                        xts[kc][:, t * P : (t + 1) * P],
                                        mts[kc][:, cc * MM_N : (cc + 1) * MM_N],
                                        start=(kc == 0),
                                        stop=False,
                                        skip_group_check=True,
                                    )
                            for cc in range(cc_per_strip):
                                nc.tensor.matmul(
                                    ps[:, cc * MM_N : (cc + 1) * MM_N],
                                    ones2,
                                    m2sb[:, s * STRIP + cc * MM_N : s * STRIP + (cc + 1) * MM_N],
                                    start=False,
                                    stop=True,
                                    skip_group_check=True,
                                )
                        if not skip_max:
                            nc.vector.max(
                                out=stash[:, (t * ns + s) * 8 : (t * ns + s + 1) * 8],
                                in_=ps,
                            )

            for t in range(qt):
                o = opool.tile([P, 8], f32, tag="o8")
                if skip_max:
                    nc.vector.memset(o, 0.0)
                elif ns > 1:
                    nc.vector.max(out=o, in_=stash[:, t * ns * 8 : (t + 1) * ns * 8])
                else:
                    nc.vector.tensor_copy(out=o, in_=stash[:, t * 8 : (t + 1) * 8])
                nc.sync.dma_start(out=c8[t], in_=o)

    return nc


def _host_inputs(phi_p, memory_bank):
    """Build per-core input maps."""
    x = np.ascontiguousarray(phi_p.reshape(B, C, H * W))  # [4, 512, 4096]
    mT = np.ascontiguousarray(memory_bank.T).astype(ml_dtypes.bfloat16)
    m2 = (memory_bank.astype(np.float64) ** 2).sum(axis=1)
    m2n = (-0.5 * m2).astype(np.float32)
    m2_hi = m2n.astype(ml_dtypes.bfloat16)
    m2_lo = (m2n - m2_hi.astype(np.float32)).astype(ml_dtypes.bfloat16)
    m2duo = np.stack([m2_hi, m2_lo], axis=0)  # [2, N_BANK]
    in_maps = []
    for i in range(N_CORES):
        b = i // 2
        lo = (i % 2) * Q_PER_CORE
        xT_i = np.ascontiguousarray(x[b][:, lo : lo + Q_PER_CORE]).astype(
            ml_dtypes.bfloat16
        )
        in_maps.append({"xT": xT_i, "mT": mT, "m2duo": m2duo})
    return in_maps


def _finish_loss(phi_p, r, c8_all):
    """c8_all: [16384, 8] top-8 of (dot - 0.5||m||^2), descending."""
    x2 = (phi_p.astype(np.float64) ** 2).sum(axis=1).reshape(Q_TOTAL)  # (b, hw) order
    d2 = x2[:, None] - 2.0 * c8_all[:, : K + J].astype(np.float64)  # ascending
    d2 = np.maximum(d2, 0.0)
    r2 = float(r[0]) ** 2
    loss_att = np.mean(np.maximum(d2[:, :K] - r2, 0.0)) / NU
    loss_rep = np.mean(np.maximum(r2 - d2[:, J:] - ALPHA, 0.0)) / NU
    return np.array(loss_att + loss_rep, dtype=np.float32)


_RESULTS_CACHE = {}


def run_device(in_maps, trace=False):
    from concourse.bass_utils import run_bass_kernel_spmd

    nc = build_program()
    if not nc.is_finalized():
        nc.finalize()
    return run_bass_kernel_spmd(
        nc, in_maps, list(range(N_CORES)), trace=trace
    )


def kernel(phi_p, memory_bank, r):
    in_maps = _host_inputs(phi_p, memory_bank)
    res = run_device(in_maps)
    c8_all = np.concatenate(
        [np.asarray(res.results[i]["c8"]).reshape(Q_PER_CORE, 8) for i in range(N_CORES)],
        axis=0,
    )
    return _finish_loss(phi_p, r, c8_all)
